# revision 56
# baseline (speedup 1.0000x reference)
"""GQA kernel for trn2, 8 NeuronCores.

Problem: B=1, S=2048, D=128, H=32, KVH=8, REP=4, rope(theta=1e4) on k AND v,
softmax(q@k^T/sqrt(128)) @ v, out @ Wo + bo.  The reference replicates torch
.view() semantics: (B,S,H*D) -> (B,H,S,D) is a FLAT reinterpretation, so
q-head h is rows [h*64,(h+1)*64) of the projection output reinterpreted as
(2048,128), and kv-head g is rows [g*256,(g+1)*256) of the k/v projections.

Sharding: core c owns kv-head g=c and q-heads {c, c+8, c+16, c+24}.
Device keeps everything in matmul-natural "storage order": q-position
j_q = b*64+a  <-> actual s' = 32a+b, kv-position j_k = b*256+a <-> t = 8a+b.
RoPE tables are host-permuted into storage order; host un-permutes rows of
the final output and sums partials over cores (Wo is a per-head row-block
contraction, so per-core partials add).

Dataflow per head: scores^T[jk,jq] = (KTr2 slice).T @ QT slice (bf16).  Q is
pre-scaled on the host by log2(e)/sqrt(128) so scores arrive in log2 units;
softmax exponentials are then 2^t with |t| < ~0.5.  13 of 16 exp tiles per
group run on ScalarE (Exp with scale=ln2); 3 run on the DVE via a custom
cubic-polynomial 2^t op (EXP2_POLY_ANT, ~0.09% max err), which keeps the
ScalarE off the critical path.  AV accumulates in psum over jk tiles (bf16
matmuls); denominators accumulate ELEMENTWISE on the DVE in bf16 and get
their cross-partition sum from a single all-ones matmul pair per group.
Fast-reciprocal + normalize on DVE, per-head Wo matmuls accumulate output
tiles.  The jk loop is software-pipelined: scores are issued two tiles
ahead of the av matmuls so the PE never waits on exp.  Inputs arrive as
packed bf16 blobs to minimize DMA-issue serialization; output leaves in
512-col chunks overlapped with the tail of compute.
"""

import re
import sys

sys.path.insert(0, "/opt/trn_rl_repo")

import numpy as np
import ml_dtypes

import concourse.bass as bass
import concourse.mybir as mybir
import concourse.tile as tile
from concourse import bacc
from concourse.bass_utils import run_bass_kernel_spmd

F32 = mybir.dt.float32
BF16 = mybir.dt.bfloat16

B, S, D = 1, 2048, 128
H, KVH, REP = 32, 8, 4
NCORES = 8
SCALE = 1.0 / np.sqrt(128.0)
LOG2E = float(np.log2(np.e))
LN2 = float(np.log(2.0))
ALPHA = SCALE * LOG2E          # host pre-scale on the Q path
ROPE_THETA = 10000.0

# minimax-ish cubic for 2^t on [-0.75, 0.75]; rel err < 9e-4
EXP2_C = (0.6939524071601757, 0.24399393291561436, 0.053601179805102726)
DVE_EXP_TILES = (11, 14)       # jk tiles whose exp runs on the DVE

# storage-order <-> position permutations
_j = np.arange(S)
PERM_Q = 32 * (_j % 64) + _j // 64          # s' = PERM_Q[j_q]
PERM_K = 8 * (_j % 256) + _j // 256         # t  = PERM_K[j_k]

# blobKW: kT(256) wk(1024); blobVW: vT(256) wv(1024); bvon: bv(1024)+onesr(128)
KW_END = 1280
VW_END = 1280
BVON_END = 1152
# blobK / blobV: cos|sin rope tables (2048 each)
BK_END = 4096
BV_END = 4096
# blob2: qT(256) wq(4096) wo(512)
B2_QT, B2_WQ, B2_WO, B2_END = 0, 256, 4352, 4864

_nc_cache = {}


def _get_exp2_op():
    """Register (once) and return the custom DVE 2^t cubic op."""
    import concourse.dve_ops as dve_ops
    for op in dve_ops.OPS:
        if op.name == "EXP2_POLY_ANT":
            return op
    from concourse.dve_spec import Spec, Src0, C0, C1, C2, One, relu
    from concourse.dve_table_gen import dve_ver_for

    def _ref(in0, in1, c0, c1, c2):
        p = 1.0 + in0 * (c0 + in0 * (c1 + in0 * c2))
        return np.maximum(p, 0.0).astype(np.float32)

    body = relu(One + Src0 * (C0 + Src0 * (C1 + Src0 * C2)))
    op = dve_ops.DveOp("EXP2_POLY_ANT", Spec(body=body, reference=_ref),
                       subdim=False, uops_sha={})
    dve_ops.OPS.append(op)
    dve_ops._SUB_OPCODE_FOR_NAME[op.name] = (
        dve_ops._CUSTOM_DVE_ROW_BASE + len(dve_ops.OPS) - 1)
    ver = dve_ver_for("TRN2")
    try:
        op.compile(ver)
    except ValueError as e:
        sha = re.search(r'="([0-9a-f]+)"', str(e)).group(1)
        op.uops_sha[ver] = sha
    op.compile(ver)
    return op


def _rope_tables():
    inv_freq = 1.0 / (ROPE_THETA ** (np.arange(0, D, 2, dtype=np.float64) / D))
    ang = np.arange(S, dtype=np.float64)[:, None] * inv_freq  # (S, 64)
    cos = np.cos(ang)  # (S, 64), same for d and d+64
    sin = np.sin(ang)

    # K-transposed layout [d, j]: value at (d, j) uses t = PERM_K[j]
    cosK = np.empty((D, S), np.float32)
    sinKe = np.empty((D, S), np.float32)
    t = PERM_K
    cosK[:64, :] = cos[t, :].T
    cosK[64:, :] = cos[t, :].T
    sinKe[:64, :] = -sin[t, :].T   # rot[d<64] = -x[d+64]
    sinKe[64:, :] = sin[t, :].T    # rot[d>=64] = +x[d-64]

    # V row layout [p, m*128+d]: row j = m*128+p, t = PERM_K[j]
    cosVr = np.empty((128, S), np.float32)
    sinVe = np.empty((128, S), np.float32)
    for m in range(16):
        tj = PERM_K[m * 128 + np.arange(128)]
        c = cos[tj, :]  # (128, 64)
        s_ = sin[tj, :]
        cosVr[:, m * 128:m * 128 + 64] = c
        cosVr[:, m * 128 + 64:m * 128 + 128] = c
        sinVe[:, m * 128:m * 128 + 64] = -s_
        sinVe[:, m * 128 + 64:m * 128 + 128] = s_
    return cosK, sinKe, cosVr, sinVe


def _build_nc():
    exp2 = _get_exp2_op()
    nc = bacc.Bacc(None)
    dp = nc.declare_dram_parameter
    blobKW = dp("blobKW", [128, KW_END], BF16, isOutput=False)
    blobVW = dp("blobVW", [128, VW_END], BF16, isOutput=False)
    bvon = dp("bvon", [1, BVON_END], BF16, isOutput=False)
    blobK = dp("blobK", [128, BK_END], BF16, isOutput=False)
    blob2 = dp("blob2", [128, B2_END], BF16, isOutput=False)
    blobV = dp("blobV", [128, BV_END], BF16, isOutput=False)
    bqk = dp("bqk", [128, 40], F32, isOutput=False)
    out = dp("out", [128, S], F32, isOutput=True)

    ADD = mybir.AluOpType.add
    MUL = mybir.AluOpType.mult
    EXP = mybir.ActivationFunctionType.Exp

    with tile.TileContext(nc) as tc:
        with tc.tile_pool(name="cst", bufs=1) as cst, \
             tc.tile_pool(name="big", bufs=1) as big, \
             tc.tile_pool(name="pb", bufs=8) as pb, \
             tc.tile_pool(name="rc", bufs=2) as rc, \
             tc.tile_pool(name="dna", bufs=4) as dna, \
             tc.tile_pool(name="psA", bufs=1, space="PSUM") as psA, \
             tc.tile_pool(name="psD", bufs=1, space="PSUM") as psD, \
             tc.tile_pool(name="psC", bufs=2, space="PSUM") as psC:
            # ---- load inputs: packed bf16 blobs + biases, in need order ----
            bkw = cst.tile([128, KW_END], BF16, tag="bkw")
            bqk_sb = cst.tile([128, 40], F32, tag="bqk")
            bvw = cst.tile([128, VW_END], BF16, tag="bvw")
            bvo = cst.tile([1, BVON_END], BF16, tag="bvo")
            bk_t = cst.tile([128, BK_END], BF16, tag="bkt")
            b2 = cst.tile([128, B2_END], BF16, tag="b2")
            bv_t = cst.tile([128, BV_END], BF16, tag="bvt")
            nc.sync.dma_start(out=bkw[:], in_=blobKW[:])
            nc.sync.dma_start(out=bvo[:], in_=bvon[:])
            nc.sync.dma_start(out=bqk_sb[:], in_=bqk[:])
            nc.sync.dma_start(out=bvw[:], in_=blobVW[:])
            nc.sync.dma_start(out=bk_t[:], in_=blobK[:])
            nc.sync.dma_start(out=b2[:], in_=blob2[:])
            nc.sync.dma_start(out=bv_t[:], in_=blobV[:])
            kT_sb = bkw[:, 0:256]
            wk_sb = bkw[:, 256:1280]
            vT_sb = bvw[:, 0:256]
            wv_sb = bvw[:, 256:1280]
            bv_sb = bvo[:, 0:1024]
            onesr_sb = bvo[:, 1024:1152]
            cosK_sb = bk_t[:, 0:2048]
            sinK_sb = bk_t[:, 2048:4096]
            qT_sb = b2[:, B2_QT:B2_QT + 256]
            wq_sb = b2[:, B2_WQ:B2_WQ + 4096]
            wo_sb = b2[:, B2_WO:B2_WO + 512]
            cosV_sb = bv_t[:, 0:2048]
            sinV_sb = bv_t[:, 2048:4096]
            bq_sb = bqk_sb[:, 0:32]
            bk_sb = bqk_sb[:, 32:40]

            # bf16 all-ones for the denominator broadcast matmul
            onesp = big.tile([128, 128], BF16, tag="onesp")
            nc.gpsimd.memset(onesp[:], 1.0)



            # ---- K projection + rope: KTr2[d, jk] ----
            KT = big.tile([128, S], BF16, tag="KT")
            for b in range(8):
                pk = psC.tile([128, 256], F32, tag="sc")
                nc.tensor.matmul(pk[:], wk_sb[:, b * 128:(b + 1) * 128],
                                 kT_sb, start=True, stop=True)
                if b % 2 == 0:
                    nc.vector.tensor_scalar(KT[:, b * 256:(b + 1) * 256],
                                            pk[:], bk_sb[:, b:b + 1],
                                            None, ADD)
                else:
                    nc.scalar.add(KT[:, b * 256:(b + 1) * 256], pk[:],
                                  bk_sb[:, b:b + 1])
            tmpK = big.tile([128, S], BF16, tag="tmpK")
            nc.vector.tensor_copy(tmpK[0:64, :], KT[64:128, :])
            nc.vector.tensor_copy(tmpK[64:128, :], KT[0:64, :])
            nc.vector.tensor_tensor(tmpK[:], tmpK[:], sinK_sb, MUL)
            nc.vector.tensor_tensor(KT[:], KT[:], cosK_sb, MUL)
            nc.vector.tensor_tensor(KT[:], KT[:], tmpK[:], ADD)

            # ---- V projection + rope in row layout: V_r2[p, m, d] ----
            VR = big.tile([128, S], BF16, tag="VR")
            vr4 = VR[:].rearrange("p (b two d) -> p b two d", b=8, two=2)
            for bg in range(2):
                for ah in range(2):
                    pv = psC.tile([128, 512], F32, tag="sc")
                    nc.tensor.matmul(pv[:], onesr_sb,
                                     bv_sb[:, bg * 512:(bg + 1) * 512],
                                     start=True, stop=False)
                    nc.tensor.matmul(pv[:], vT_sb[:, ah * 128:(ah + 1) * 128],
                                     wv_sb[:, bg * 512:(bg + 1) * 512],
                                     start=False, stop=True,
                                     skip_group_check=True)
                    if ah == 0:
                        nc.vector.tensor_copy(
                            vr4[:, 4 * bg:4 * bg + 4, ah, :],
                            pv[:].rearrange("p (b d) -> p b d", b=4))
                    else:
                        nc.scalar.copy(
                            vr4[:, 4 * bg:4 * bg + 4, ah, :],
                            pv[:].rearrange("p (b d) -> p b d", b=4))
            tmpV = big.tile([128, S], BF16, tag="tmpV")
            vr3 = VR[:].rearrange("p (m h d) -> p m h d", m=16, h=2)
            tv3 = tmpV[:].rearrange("p (m h d) -> p m h d", m=16, h=2)
            sv3 = sinV_sb.rearrange("p (m h d) -> p m h d", m=16, h=2)
            nc.vector.tensor_tensor(tv3[:, :, 0, :], vr3[:, :, 1, :],
                                    sv3[:, :, 0, :], MUL)
            nc.vector.tensor_tensor(tv3[:, :, 1, :], vr3[:, :, 0, :],
                                    sv3[:, :, 1, :], MUL)
            nc.vector.tensor_tensor(VR[:], VR[:], cosV_sb, MUL)
            nc.vector.tensor_tensor(VR[:], VR[:], tmpV[:], ADD)
            vr2t = VR[:].rearrange("p (m d) -> p m d", m=16)

            # ---- Q projection: QT_all[d, h, b, a]; alternate the psum->sbuf
            # bias-add between DVE and ScalarE.  Blocks 16..31 are only
            # needed by the half=1 attention groups, so they are deferred to
            # the half boundary (shorter head). ----
            QT = big.tile([128, 4 * S], BF16, tag="QT")
            qt4 = QT[:].rearrange("p (h b a) -> p h b a", h=4, b=32)

            def emit_qproj(b, pool=None):
                pq = (pool or psC).tile([128, 256], F32,
                                        tag="sc" if pool is None else "aux",
                                        name="pq")
                nc.tensor.matmul(pq[:], wq_sb[:, b * 128:(b + 1) * 128],
                                 qT_sb, start=True, stop=True)
                # head blocks (b<16) all copy via ScalarE: the DVE is busy
                # with rope there and would stall the projection; deferred
                # blocks alternate since both engines are loaded then.
                if b >= 16 and b % 2 == 0:
                    nc.vector.tensor_scalar(
                        qt4[:, :, b, :],
                        pq[:].rearrange("p (h a) -> p h a", h=4),
                        bq_sb[:, b:b + 1], None, ADD)
                else:
                    nc.scalar.add(qt4[:, :, b, :],
                                  pq[:].rearrange("p (h a) -> p h a", h=4),
                                  bq_sb[:, b:b + 1])

            for b in range(16):
                emit_qproj(b)

            # ---- attention per head (software-pipelined over jk) ----
            # PSUM budget: av(2) + dnbc(2) + 2 in-flight sc tiles (4) = 8.
            OHT = big.tile([128, 4 * S], BF16, tag="OHT")
            out_sb = big.tile([128, S], F32, tag="osb")
            tails = []

            def new_gstate(h, half):
                av = psA.tile([128, 1024], F32, tag="av", name="av")
                dnacc = dna.tile([128, 1024], BF16, tag="dnacc",
                                 name="dnacc")
                return {"base": h * S + half * 1024, "av": av,
                        "dnacc": dnacc, "prs": {}, "dnbc": []}

            def emit_sc(gs, jk):
                on_dve = jk in DVE_EXP_TILES
                sc = (psD if on_dve else psC).tile(
                    [128, 1024], F32,
                    tag="aux" if on_dve else "sc", name="sc")
                base = gs["base"]
                for c in range(2):
                    nc.tensor.matmul(
                        sc[:, c * 512:(c + 1) * 512],
                        KT[:, jk * 128:(jk + 1) * 128],
                        QT[:, base + c * 512:base + (c + 1) * 512],
                        start=True, stop=True)
                pr = pb.tile([128, 1024], BF16, tag="pr", name="pr")
                if on_dve:
                    nc.vector._custom_dve(exp2, out=pr[:], in0=sc[:],
                                          s0=EXP2_C[0], s1=EXP2_C[1],
                                          imm2=EXP2_C[2])
                else:
                    nc.scalar.activation(pr[:], sc[:], EXP, scale=LN2)
                gs["prs"][jk] = pr

            def emit_avd(gs, jk):
                pr = gs["prs"].pop(jk)
                av = gs["av"]
                for c in range(2):
                    cs = slice(c * 512, (c + 1) * 512)
                    nc.tensor.matmul(av[:, cs], vr2t[:, jk, :], pr[:, cs],
                                     start=(jk == 0), stop=(jk == 15),
                                     skip_group_check=True)
                if jk == 15:
                    gs["dnbc"].append(psD.tile([128, 1024], F32,
                                               tag="aux", name="dnbc"))
                    for c in range(2):
                        cs = slice(c * 512, (c + 1) * 512)
                        nc.tensor.matmul(gs["dnbc"][0][:, cs], onesp[:],
                                         pr[:, cs], start=True, stop=False,
                                         skip_group_check=True)
                elif jk == 0:
                    nc.vector.tensor_copy(gs["dnacc"][:], pr[:])
                else:
                    nc.vector.tensor_tensor(gs["dnacc"][:], gs["dnacc"][:],
                                            pr[:], ADD)

            def emit_group(gs, nxt, extras=()):
                # gs: this group's state (its sc(0)/sc(1) were emitted by
                # the previous group's jk 13/14, or by the caller for the
                # first group); nxt: next group's state — its first two
                # scores tiles are emitted here so the ScalarE exp stream
                # never stalls at the group boundary.
                extras = list(extras)
                if tails:
                    tails.pop(0)()
                for jk in range(16):
                    emit_avd(gs, jk)
                    if extras and jk >= 1:
                        extras.pop(0)()
                    if jk + 2 < 16:
                        emit_sc(gs, jk + 2)
                    if nxt is not None and jk == 13:
                        emit_sc(nxt, 0)
                    elif nxt is not None and jk == 14:
                        emit_sc(nxt, 1)

                def tail(gs=gs):
                    dnbc = gs["dnbc"][0]
                    for c in range(2):
                        cs = slice(c * 512, (c + 1) * 512)
                        nc.tensor.matmul(dnbc[:, cs], onesp[:],
                                         gs["dnacc"][:, cs],
                                         start=False, stop=True,
                                         skip_group_check=True)
                    rcp = rc.tile([128, 1024], F32, tag="rcp", name="rcp")
                    nc.vector.reciprocal_approx_fast(rcp[:], dnbc[:])
                    nc.vector.tensor_tensor(
                        OHT[:, gs["base"]:gs["base"] + 1024],
                        gs["av"][:], rcp[:], MUL)

                tails.append(tail)

            def emit_wo(jc):
                po = psD.tile([128, 512], F32, tag="aux", name="po")
                for h in range(4):
                    nc.tensor.matmul(po[:],
                                     wo_sb[:, h * 128:(h + 1) * 128],
                                     OHT[:, h * S + jc * 512:
                                         h * S + (jc + 1) * 512],
                                     start=(h == 0), stop=(h == 3),
                                     skip_group_check=True)
                nc.vector.tensor_copy(out_sb[:, jc * 512:(jc + 1) * 512],
                                      po[:])
                nc.sync.dma_start(out=out[:, jc * 512:(jc + 1) * 512],
                                  in_=out_sb[:, jc * 512:(jc + 1) * 512])

            # half-outer group order; deferred Q-projection blocks and Wo
            # chunks are interleaved into group bodies so no engine sits
            # idle at the half boundary.
            order = [(0, 0), (1, 0), (2, 0), (3, 0),
                     (0, 1), (1, 1), (2, 1), (3, 1)]
            group_extras = {
                (2, 0): [lambda b=b: emit_qproj(b, psD)
                         for b in range(16, 24)],
                (3, 0): [lambda b=b: emit_qproj(b, psD)
                         for b in range(24, 32)],
                (0, 1): [lambda: emit_wo(0)],
                (1, 1): [lambda: emit_wo(1)],
            }
            gstates = [None] * 8
            gstates[0] = new_gstate(*order[0])
            emit_sc(gstates[0], 0)
            emit_sc(gstates[0], 1)
            for gi in range(8):
                if gi + 1 < 8:
                    gstates[gi + 1] = new_gstate(*order[gi + 1])
                    nxt = gstates[gi + 1]
                else:
                    nxt = None
                emit_group(gstates[gi], nxt,
                           group_extras.get(order[gi], ()))
            tails.pop(0)()
            emit_wo(2)
            emit_wo(3)

    nc.compile()
    return nc


def _get_nc():
    if "nc" not in _nc_cache:
        _nc_cache["nc"] = _build_nc()
    return _nc_cache["nc"]


def make_in_maps(query, keys, values, Wq, bq, Wk, bk, Wv, bv, Wo, bo):
    BF = ml_dtypes.bfloat16
    cosK, sinKe, cosVr, sinVe = _rope_tables()
    q2 = np.asarray(query, np.float32).reshape(S, D) * ALPHA
    k2 = np.asarray(keys, np.float32).reshape(S, D)
    v2 = np.asarray(values, np.float32).reshape(S, D)
    Wq_ = np.ascontiguousarray(np.asarray(Wq, np.float32))
    Wk_ = np.ascontiguousarray(np.asarray(Wk, np.float32))
    Wv_ = np.ascontiguousarray(np.asarray(Wv, np.float32))
    Wo_ = np.asarray(Wo, np.float32)
    bq_ = np.asarray(bq, np.float32).reshape(32, 128).T.copy() * ALPHA
    bk_ = np.asarray(bk, np.float32).reshape(8, 128).T.copy()
    bv_ = np.asarray(bv, np.float32).reshape(1, KVH * D)

    bqk = np.zeros((128, 40), np.float32)
    bqk[:, 0:32] = bq_
    bqk[:, 32:40] = bk_

    blobK = np.empty((128, BK_END), BF)
    blobK[:, 0:2048] = cosK.astype(BF)
    blobK[:, 2048:4096] = sinKe.astype(BF)
    blobV = np.empty((128, BV_END), BF)
    blobV[:, 0:2048] = cosVr.astype(BF)
    blobV[:, 2048:4096] = sinVe.astype(BF)

    in_maps = []
    for c in range(NCORES):
        heads = [c + 8 * r for r in range(REP)]
        qrows = np.concatenate([q2[hh * 64:(hh + 1) * 64] for hh in heads])
        woc = np.concatenate([Wo_[hh * 128:(hh + 1) * 128] for hh in heads],
                             axis=1)  # [128, 4*128]
        blobKW = np.empty((128, KW_END), BF)
        blobKW[:, 0:256] = k2[c * 256:(c + 1) * 256].T.astype(BF)
        blobKW[:, 256:1280] = Wk_.astype(BF)
        blobVW = np.empty((128, VW_END), BF)
        blobVW[:, 0:256] = v2[c * 256:(c + 1) * 256].T.astype(BF)
        blobVW[:, 256:1280] = Wv_.astype(BF)
        bvon = np.empty((1, BVON_END), BF)
        bvon[0, 0:1024] = bv_[0].astype(BF)
        bvon[0, 1024:1152] = np.ones(128, BF)
        blob2 = np.empty((128, B2_END), BF)
        blob2[:, B2_QT:B2_QT + 256] = qrows.T.astype(BF)
        blob2[:, B2_WQ:B2_WQ + 4096] = Wq_.astype(BF)
        blob2[:, B2_WO:B2_WO + 512] = woc.astype(BF)
        in_maps.append({
            "blobKW": blobKW, "blobVW": blobVW, "bvon": bvon,
            "blobK": blobK, "blob2": blob2, "blobV": blobV, "bqk": bqk,
        })
    return in_maps


def kernel(query, keys, values, Wq, bq, Wk, bk, Wv, bv, Wo, bo):
    nc = _get_nc()
    in_maps = make_in_maps(query, keys, values, Wq, bq, Wk, bk, Wv, bv, Wo, bo)
    res = run_bass_kernel_spmd(nc, in_maps, list(range(NCORES)))
    return postprocess(res.results, bo)


def postprocess(results, bo):
    acc = np.zeros((S, D), np.float64)
    for c in range(NCORES):
        o = np.asarray(results[c]["out"], np.float32)  # [dout=128, jq=2048]
        acc += o.T
    final = np.empty((S, D), np.float32)
    final[PERM_Q] = acc.astype(np.float32)
    final += np.asarray(bo, np.float32)
    return final.reshape(B, S, D)
